# revision 36
# baseline (speedup 1.0000x reference)
"""Trainium2 Bass kernel for DynamicSparseAttention.

Reference computation (per batch b, head h):
    scores  = Q @ K^T                      [L, S]
    dense   = softmax(scores, axis=-1)
    routing = dense ** 5
    combined = (routing + dense) * 0.5
    sparse  = combined / sum(combined, -1, keepdims=True)
    out     = sparse @ V                   [L, D]

Math: let p = exp(s - m1) with a per-row analytic upper bound m1 and
Z = sum_s p.  d = p/Z is the exact softmax, and with V' = [V | 1]:
    num = D5 @ V' + (P @ V')/Z   (rows 0..63 numerator, row 64 denominator)
    out = num[:, :D] / num[:, D]

Round A (per l-half): one fp16 score matmul stream (64 q/k channels plus a
ones*(-m1) shift channel, fp32 PSUM accumulate), exp on ACT -> p (bf16,
kept in SBUF), and B = P @ V' accumulated on the PE (ones column gives Z).

Round B builds p5 = d^5 per 128-row s-tile via two engine paths, balancing
the ACT and DVE engines (the two all-stock bottlenecks):
  - DVE path (N_DVE tiles):  d = p*zb (zb = broadcast bf16 1/Z), then
    d2 = d*d, d4 = d2*d2, p5 = d4*d  -- four 2x-mode tensor_tensor ops.
  - ACT path (rest): recompute scores with one extra channel
    (-1 ones row) * (lnZ row written on-device by an ACT Ln, fp16), then
    p5 = exp(5 * (s - m1 - lnZ)) in a single ACT pass.
Both produce the same d^5 scale, accumulated into A5 = P5 @ V'.

zb = broadcast bf16(1/Z) is exp(-lnZ) on ACT (a DVE reciprocal over
[1,1024] is an 8-pass iterative divide, ~7us) replicated across partitions
by a K=1 PE matmul against a ones row.

Epilogue: num = A5 + B*(1/Z) (two DVE tensor_tensor ops straight from
PSUM/SBUF), then per 128 rows: PE transpose, DVE reciprocal of the
denominator, ACT scaled copy, DMA out.

The (pair, l-half) units are software-pipelined depth 1, emitted as
  [B_act(i-1) | A(i) | Ln/zr/b_sb(i) | B_rest_mms(i-1) | zp(i) | B_epi(i-1)]
so each engine queue stays busy: the exp5s of unit i-1 sit ahead of unit
i's exps on ACT; the DVE chains of i-1 run during A(i)'s PE/ACT work and
are not queued behind mid(i)'s zb cast (which waits on late-A(i) PE work);
the zp replicate matmuls cover the PE's wait on the num add; and the
epilogue transposes allocate from the ps_acc ring (whose two slots are both
dead by then) instead of the score ring, which otherwise serializes the
next unit's score matmuls against this unit's epilogue.  Within A, PV-B for
s-tile st is emitted after scores+exp of st+1 so the PE never head-blocks
on an exp.  The last unit's round B is mostly-ACT (its DVE chains would
drain serially with no next unit to overlap).  PSUM: 2 score buffers
(4 banks) + 2 accumulators (4 banks) = 8 banks exactly; Bacc is freed
early via an ACT copy to SBUF (GPSIMD cannot read PSUM).

Numerics: rel err ~4.6e-3 on hardware (gate 2e-2).

Sharding: B*H = 32 (b,h) pairs, 4 per core across 8 cores, no cross-core
communication.  kernel() takes full inputs and returns the full output.
Q/K are pre-transposed on the host and DMA'd in [66, L] layout.
"""

import os
import sys
import numpy as np

for _p in ("/opt/trn_rl_repo",):
    if os.path.isdir(_p) and _p not in sys.path:
        sys.path.insert(0, _p)

from contextlib import ExitStack

import json as _json

import ml_dtypes

import concourse.bass as bass
import concourse.mybir as mybir
import concourse.tile as tile
import concourse.bass2jax as _bass2jax
import concourse.bass_utils as _bass_utils
from concourse.bass_utils import run_bass_kernel_spmd
from concourse.masks import make_identity

# ---------------------------------------------------------------------------
# Workaround: this container's walrus build rejects instructions carrying
# more than one sync wait ("Too many sync wait commands").  Tile's scheduler
# freely attaches 2-3 waits per instruction.  Rewrite the BIR JSON before
# compilation: excess waits are hoisted onto freshly inserted same-engine
# NoOp instructions placed immediately before the instruction, one wait
# each.  Semantics are unchanged (waits are conjunctive >= conditions and
# engine program order is preserved).
# ---------------------------------------------------------------------------

_MAX_WAITS = 1


def _split_waits_in_bir(bir_json: bytes) -> bytes:
    bir = _json.loads(bir_json)
    n_new = [0]

    def fix_block(bb):
        out = []
        for inst in bb["instructions"]:
            si = inst.get("sync_info") or {}
            waits = si.get("on_wait") or []
            if len(waits) > _MAX_WAITS:
                excess, keep = waits[:-_MAX_WAITS], waits[-_MAX_WAITS:]
                for w in excess:
                    n_new[0] += 1
                    out.append({
                        "debug": inst.get("debug", 0),
                        "engine": inst["engine"],
                        "ins": [],
                        "name": "I-wsplit-%d" % n_new[0],
                        "opcode": "NoOp",
                        "outs": [],
                        "sync_info": {"on_update": [], "on_wait": [w]},
                    })
                si["on_wait"] = keep
            out.append(inst)
        bb["instructions"] = out

    for fn in bir["functions"]:
        for bb in fn["blocks"]:
            fix_block(bb)
    return _json.dumps(bir).encode()


_orig_compile_bir_kernel = _bass_utils.compile_bir_kernel


def _patched_compile_bir_kernel(bir_json, tmpdir, neff_name="file.neff"):
    return _orig_compile_bir_kernel(
        _split_waits_in_bir(bir_json), tmpdir, neff_name=neff_name
    )


_bass_utils.compile_bir_kernel = _patched_compile_bir_kernel
_bass2jax.compile_bir_kernel = _patched_compile_bir_kernel

# (walrus's --enable-ldw-opt dedup is incompatible with the framework's
# explicit InstLdweights preamble on this build; leave it off.)

# ---------------------------------------------------------------------------

B, L, S, H, E, D = 2, 2048, 2048, 16, 64, 64
NCORES = 8
NP = (B * H) // NCORES  # pairs per core = 4
EC = E + 2   # channels: 64 fp16 q/k + ones*(-m1) + (-1)*lnZ
DV = D + 1   # v columns: 64 data + ones column (carries Z / denominator)
LHALF = 1024
NCH = 2      # 512-wide matmul chunks (PSUM bank limit)
NLH = L // LHALF
ST = S // 128
N_DVE = 10   # s-tiles per l-half whose p5 is computed on the DVE
FACTOR = 5.0

F32 = mybir.dt.float32
BF16 = mybir.dt.bfloat16
FP16 = mybir.dt.float16
EXP = mybir.ActivationFunctionType.Exp
LN = mybir.ActivationFunctionType.Ln
COPY = mybir.ActivationFunctionType.Copy
MULT = mybir.AluOpType.mult
ADD = mybir.AluOpType.add

M_COEF = float(np.sqrt(2.0 * np.log(S)))
M_MARGIN = 25.0


def _emit(ctx: ExitStack, tc: tile.TileContext, qtd, ktd, vad, outp):
    nc = tc.nc

    const = ctx.enter_context(tc.tile_pool(name="const", bufs=1))
    kq = ctx.enter_context(tc.tile_pool(name="kq", bufs=2))
    vpool = ctx.enter_context(tc.tile_pool(name="vp", bufs=2))
    ppool = ctx.enter_context(tc.tile_pool(name="pp", bufs=2))
    p5pool = ctx.enter_context(tc.tile_pool(name="p5", bufs=4))
    dpool = ctx.enter_context(tc.tile_pool(name="dp", bufs=3))
    zpool = ctx.enter_context(tc.tile_pool(name="zp", bufs=2))
    eppool = ctx.enter_context(tc.tile_pool(name="ep", bufs=3))
    opool = ctx.enter_context(tc.tile_pool(name="op", bufs=4))

    ps_sc = ctx.enter_context(tc.tile_pool(name="ps_sc", bufs=2, space="PSUM"))
    ps_acc = ctx.enter_context(tc.tile_pool(name="ps_acc", bufs=2, space="PSUM"))

    ident65 = const.tile([DV, DV], F32)
    make_identity(nc, ident65)
    identb65 = const.tile([DV, DV], BF16)
    make_identity(nc, identb65)
    ones_row = const.tile([1, 128], BF16)
    nc.vector.memset(ones_row, 1.0)

    # per-pair tiles, loaded once (ring 2 across pairs)
    loaded = {}

    def load_pair(bh):
        qtT = kq.tile([EC, L], FP16, tag="qt", name="qtT")
        nc.sync.dma_start(out=qtT, in_=qtd[bh])
        ktT = kq.tile([EC, S], FP16, tag="kt", name="ktT")
        nc.sync.dma_start(out=ktT, in_=ktd[bh])
        vts = []
        for t in range(ST):
            vt = vpool.tile([128, DV], BF16, tag=f"v{t}", name=f"vt{t}")
            nc.sync.dma_start(out=vt, in_=vad[bh, t * 128:(t + 1) * 128, :])
            vts.append(vt)
        loaded[bh] = (qtT, ktT, vts)

    def emit_A(bh, lh):
        qtT, ktT, vts = loaded[bh]
        l0 = lh * LHALF
        bacc = ps_acc.tile([DV, LHALF], F32, tag="acc", name="bacc")
        ps = []

        def emit_pvb(st):
            for c in range(NCH):
                cs = slice(c * 512, (c + 1) * 512)
                nc.tensor.matmul(bacc[:, cs], lhsT=vts[st], rhs=ps[st][:, cs],
                                 start=(st == 0), stop=(st == ST - 1))

        # PV-B for s-tile st is emitted after the scores+exp of st+1, so the
        # PE never head-blocks on the exp it just requested
        for st in range(ST):
            sb = slice(st * 128, (st + 1) * 128)
            sc = ps_sc.tile([128, LHALF], F32, tag="sc", name="scA")
            for c in range(NCH):
                cs = slice(c * 512, (c + 1) * 512)
                gs = slice(l0 + c * 512, l0 + (c + 1) * 512)
                nc.tensor.matmul(sc[:, cs], lhsT=ktT[0:E + 1, sb],
                                 rhs=qtT[0:E + 1, gs], start=True, stop=True)
            p = ppool.tile([128, LHALF], BF16, tag=f"p{st}", name="p")
            nc.scalar.activation(p, sc, EXP, bias=0.0, scale=1.0)
            ps.append(p)
            if st > 0:
                emit_pvb(st - 1)
        emit_pvb(ST - 1)

        # lnZ channel (fp16 row 65 of qt, matched with -1 ones row in kt).
        # Engine writes must start at a 32-aligned partition, so Ln lands in
        # an aligned scratch row and a tiny SBUF->SBUF DMA moves it into place.
        lz = zpool.tile([1, LHALF], FP16, tag="lz", name="lz")
        nc.scalar.activation(lz, bacc[D:DV, :], LN, bias=0.0, scale=1.0)
        nc.sync.dma_start(out=qtT[E + 1:E + 2, l0:l0 + LHALF], in_=lz)
        # zb = broadcast bf16(1/Z); 1/Z = exp(-lnZ) on ACT (a DVE reciprocal
        # over [1,1024] is an 8-pass iterative divide, ~7us -- far slower)
        zr = zpool.tile([1, LHALF], BF16, tag="zr", name="zr")
        nc.scalar.activation(zr, lz, EXP, bias=0.0, scale=-1.0)
        zp = ps_sc.tile([128, LHALF], F32, tag="sc", name="zp")
        for c in range(NCH):
            cs = slice(c * 512, (c + 1) * 512)
            nc.tensor.matmul(zp[:, cs], lhsT=ones_row, rhs=zr[:, cs],
                             start=True, stop=True)
        zb = zpool.tile([128, LHALF], BF16, tag="zb", name="zb")
        nc.vector.tensor_copy(zb, zp)
        # free bacc early (GPSIMD cannot read PSUM, so ACT does the copy)
        b_sb = eppool.tile([DV, LHALF], F32, tag="bsb", name="b_sb")
        nc.scalar.activation(b_sb, bacc, COPY, bias=0.0, scale=1.0)
        return {"ps": ps, "zb": zb, "b_sb": b_sb, "lh": lh, "bh": bh}

    N_ACT = ST - N_DVE

    def emit_B_act(stA):
        """Round-B ACT-path tiles (sts 0..n_act-1): score recompute with the
        lnZ channel + exp(5*) on ACT.  Emitted BEFORE the next unit's A-phase
        so the exp5s sit early in the ACT queue."""
        bh, lh = stA["bh"], stA["lh"]
        qtT, ktT, vts = loaded[bh]
        l0 = lh * LHALF
        a5 = ps_acc.tile([DV, LHALF], F32, tag="acc", name="a5")
        stA["a5"] = a5
        # t1 = B/Z only needs b_sb and zb (ready since this unit's mid);
        # computing it here keeps the epilogue's num = t1 + A5 one DVE op
        # after the last chain instead of two
        t1 = eppool.tile([DV, LHALF], F32, tag="t1", name="t1")
        nc.vector.tensor_tensor(out=t1, in0=stA["b_sb"], in1=stA["zb"][0:DV, :],
                                op=MULT)
        stA["t1"] = t1
        p5s = {}

        def emit_a5(st):
            for c in range(NCH):
                cs = slice(c * 512, (c + 1) * 512)
                nc.tensor.matmul(a5[:, cs], lhsT=vts[st], rhs=p5s[st][:, cs],
                                 start=(st == 0), stop=(st == ST - 1))

        for st in range(stA["n_act"]):
            sb = slice(st * 128, (st + 1) * 128)
            sc = ps_sc.tile([128, LHALF], F32, tag="sc", name="scB")
            for c in range(NCH):
                cs = slice(c * 512, (c + 1) * 512)
                gs = slice(l0 + c * 512, l0 + (c + 1) * 512)
                nc.tensor.matmul(sc[:, cs], lhsT=ktT[:, sb],
                                 rhs=qtT[:, gs], start=True, stop=True)
            p5 = p5pool.tile([128, LHALF], BF16, tag="p5a", name="p5")
            nc.scalar.activation(p5, sc, EXP, bias=0.0, scale=FACTOR)
            p5s[st] = p5
            if st > 0:
                emit_a5(st - 1)
        emit_a5(stA["n_act"] - 1)

    def emit_B_rest(stA):
        """Round-B DVE-path tiles (sts N_ACT..15) + combine + epilogue."""
        bh, lh = stA["bh"], stA["lh"]
        qtT, ktT, vts = loaded[bh]
        ps, zb, b_sb, a5 = stA["ps"], stA["zb"], stA["b_sb"], stA["a5"]
        l0 = lh * LHALF
        for st in range(stA["n_act"], ST):
            d = dpool.tile([128, LHALF], BF16, tag="d", name="d")
            nc.vector.tensor_tensor(out=d, in0=ps[st], in1=zb, op=MULT)
            d2 = dpool.tile([128, LHALF], BF16, tag="d2", name="d2")
            nc.vector.tensor_tensor(out=d2, in0=d, in1=d, op=MULT)
            d4 = dpool.tile([128, LHALF], BF16, tag="d4", name="d4")
            nc.vector.tensor_tensor(out=d4, in0=d2, in1=d2, op=MULT)
            p5 = p5pool.tile([128, LHALF], BF16, tag="p5", name="p5")
            nc.vector.tensor_tensor(out=p5, in0=d4, in1=d, op=MULT)
            for c in range(NCH):
                cs = slice(c * 512, (c + 1) * 512)
                nc.tensor.matmul(a5[:, cs], lhsT=vts[st], rhs=p5[:, cs],
                                 start=False, stop=(st == ST - 1))

        # combine: num = A5 + B/Z (one DVE op; t1 was computed in B_act)
        t1 = stA["t1"]
        num = eppool.tile([DV, LHALF], F32, tag="num", name="num")
        nc.vector.tensor_tensor(out=num, in0=t1, in1=a5, op=ADD)
        stA["num"] = num

    def emit_B_epi(stA):
        """out = num[:, :D] / num[:, D].  The nt transposes allocate from the
        ps_acc ring (both of its slots are dead here: bacc freed by the early
        b_sb copy, a5 by the num add) so the sc ring is left free for the
        next unit's score tiles."""
        bh, lh, num = stA["bh"], stA["lh"], stA["num"]
        l0 = lh * LHALF
        for ch in range(LHALF // 128):
            nt = ps_acc.tile([128, DV], F32, tag="acc", name="nt")
            nc.tensor.transpose(nt, num[:, ch * 128:(ch + 1) * 128], ident65)
            rz = opool.tile([128, 1], F32, tag="rz", name="rz")
            nc.vector.reciprocal(rz, nt[:, D:DV])
            ot = opool.tile([128, D], F32, tag="ot", name="ot")
            nc.vector.tensor_scalar_mul(ot, nt[:, 0:D], rz[:, 0:1])
            lrow = l0 + ch * 128
            nc.gpsimd.dma_start(out=outp[bh, lrow:lrow + 128, :], in_=ot)

    units = [(bh, lh) for bh in range(NP) for lh in range(NLH)]
    pending = None
    for i, (bh, lh) in enumerate(units):
        if lh == 0:
            load_pair(bh)
        if pending is not None:
            emit_B_act(pending)
        qtT, ktT, vts = loaded[bh]
        l0 = lh * LHALF
        bacc = ps_acc.tile([DV, LHALF], F32, tag="acc", name="bacc")
        ps = []

        def emit_pvb(st):
            for c in range(NCH):
                cs = slice(c * 512, (c + 1) * 512)
                nc.tensor.matmul(bacc[:, cs], lhsT=vts[st], rhs=ps[st][:, cs],
                                 start=(st == 0), stop=(st == ST - 1))

        # PV-B for s-tile st is emitted after the scores+exp of st+1, so the
        # PE never head-blocks on the exp it just requested
        for st in range(ST):
            sb = slice(st * 128, (st + 1) * 128)
            sc = ps_sc.tile([128, LHALF], F32, tag="sc", name="scA")
            for c in range(NCH):
                cs = slice(c * 512, (c + 1) * 512)
                gs = slice(l0 + c * 512, l0 + (c + 1) * 512)
                nc.tensor.matmul(sc[:, cs], lhsT=ktT[0:E + 1, sb],
                                 rhs=qtT[0:E + 1, gs], start=True, stop=True)
            p = ppool.tile([128, LHALF], BF16, tag=f"p{st}", name="p")
            nc.scalar.activation(p, sc, EXP, bias=0.0, scale=1.0)
            ps.append(p)
            if st > 0:
                emit_pvb(st - 1)
        emit_pvb(ST - 1)

        # lnZ chain early: Ln gates the lnZ DMA and thus every B_act(i) PE
        # matmul; emitting it here puts it ahead of the previous unit's
        # epilogue copies in the ACT queue.
        lz = zpool.tile([1, LHALF], FP16, tag="lz", name="lz")
        nc.scalar.activation(lz, bacc[D:DV, :], LN, bias=0.0, scale=1.0)
        nc.sync.dma_start(out=qtT[E + 1:E + 2, l0:l0 + LHALF], in_=lz)
        zr = zpool.tile([1, LHALF], BF16, tag="zr", name="zr")
        nc.scalar.activation(zr, lz, EXP, bias=0.0, scale=-1.0)
        # free bacc now: as the LAST ACT op of a cycle this stalled both the
        # next a5 ring allocation (PE) and the hoisted t1 (DVE head-of-line)
        b_sb = eppool.tile([DV, LHALF], F32, tag="bsb", name="b_sb")
        nc.scalar.activation(b_sb, bacc, COPY, bias=0.0, scale=1.0)

        # B_rest(i-1): its DVE chains are ready (zb of i-1 exists) and must
        # not queue behind mid(i)'s zb cast, which waits on late PE work.
        if pending is not None:
            emit_B_rest(pending)

        # ---- mid: zb broadcast.  The zp matmuls sit between B_rest(i-1)'s
        # A5 matmuls and its epilogue transposes in the PE queue, covering
        # the PE's wait on the num add (DVE).
        zp = ps_sc.tile([128, LHALF], F32, tag="sc", name="zp")
        for c in range(NCH):
            cs = slice(c * 512, (c + 1) * 512)
            nc.tensor.matmul(zp[:, cs], lhsT=ones_row, rhs=zr[:, cs],
                             start=True, stop=True)
        zb = zpool.tile([128, LHALF], BF16, tag="zb", name="zb")
        nc.vector.tensor_copy(zb, zp)

        if pending is not None:
            emit_B_epi(pending)

        # the last unit's round B drains with no A-phase to overlap; give it
        # all-ACT tiles so its p5 work overlaps its own PE/ACT instead of
        # serializing on the DVE chain
        # last unit: no next unit overlaps its round B, so balance its p5
        # work across ACT (exp5) and DVE (chains) instead of one engine
        n_act = N_ACT if i < len(units) - 1 else 10
        pending = {"ps": ps, "zb": zb, "b_sb": b_sb, "lh": lh, "bh": bh,
                   "n_act": n_act}
    emit_B_act(pending)
    emit_B_rest(pending)
    emit_B_epi(pending)


_CACHE = {}


def _build():
    if "nc" in _CACHE:
        return _CACHE["nc"]
    nc = bass.Bass()
    qtd = nc.declare_dram_parameter("qt", [NP, EC, L], FP16, isOutput=False)
    ktd = nc.declare_dram_parameter("kt", [NP, EC, S], FP16, isOutput=False)
    vad = nc.declare_dram_parameter("va", [NP, S, DV], BF16, isOutput=False)
    outp = nc.declare_dram_parameter("out", [NP, L, D], F32, isOutput=True)
    with tile.TileContext(nc) as tc:
        with ExitStack() as ctx:
            _emit(ctx, tc, qtd[:], ktd[:], vad[:], outp[:])
    _CACHE["nc"] = nc
    return nc


def _prep_inputs(queries, keys, values):
    bf = ml_dtypes.bfloat16
    q = np.ascontiguousarray(np.asarray(queries, np.float32)
                             .transpose(0, 2, 1, 3)).reshape(B * H, L, E)
    k = np.ascontiguousarray(np.asarray(keys, np.float32)
                             .transpose(0, 2, 1, 3)).reshape(B * H, S, E)
    v = np.ascontiguousarray(np.asarray(values, np.float32)
                             .transpose(0, 2, 1, 3)).reshape(B * H, S, D)
    m1 = (M_COEF * np.sqrt((q.astype(np.float64) ** 2).sum(-1)) + M_MARGIN
          ).astype(np.float32)  # [BH, L]
    one_s = np.ones((B * H, S, 1), np.float32)
    zero_l = np.zeros((B * H, L, 1), np.float32)
    # qt channels: q | -m1 | lnZ placeholder (written on device per l-half)
    qt = np.concatenate([q, -m1[..., None], zero_l], axis=-1)   # [., L, 66]
    # kt channels: k | +1 (pairs with -m1) | -1 (pairs with lnZ)
    kt = np.concatenate([k, one_s, -one_s], axis=-1)            # [., S, 66]
    qt = np.ascontiguousarray(qt.transpose(0, 2, 1)).astype(np.float16)
    kt = np.ascontiguousarray(kt.transpose(0, 2, 1)).astype(np.float16)
    va = np.concatenate([v.astype(bf), one_s.astype(bf)], axis=-1)
    in_maps = []
    for c in range(NCORES):
        sl = slice(c * NP, (c + 1) * NP)
        in_maps.append({
            "qt": np.ascontiguousarray(qt[sl]),
            "kt": np.ascontiguousarray(kt[sl]),
            "va": np.ascontiguousarray(va[sl]),
        })
    return in_maps


def _gather(results):
    outs = np.stack([results[c]["out"] for c in range(NCORES)])  # [8,NP,L,D]
    out = outs.reshape(B, H, L, D).transpose(0, 2, 1, 3)
    return np.ascontiguousarray(out)


def run_sharded(queries, keys, values, **kw):
    """Run on the 8 neuron cores; returns (full_output, BassKernelResults)."""
    nc = _build()
    in_maps = _prep_inputs(queries, keys, values)
    res = run_bass_kernel_spmd(nc, in_maps, list(range(NCORES)), **kw)
    return _gather(res.results), res


def kernel(queries, keys, values):
    out, _ = run_sharded(queries, keys, values)
    return out


# revision 37
# speedup vs baseline: 1.0095x; 1.0095x over previous
"""Trainium2 Bass kernel for DynamicSparseAttention.

Reference computation (per batch b, head h):
    scores  = Q @ K^T                      [L, S]
    dense   = softmax(scores, axis=-1)
    routing = dense ** 5
    combined = (routing + dense) * 0.5
    sparse  = combined / sum(combined, -1, keepdims=True)
    out     = sparse @ V                   [L, D]

Math: let p = exp(s - m1) with a per-row analytic upper bound m1 and
Z = sum_s p.  d = p/Z is the exact softmax, and with V' = [V | 1]:
    num = D5 @ V' + (P @ V')/Z   (rows 0..63 numerator, row 64 denominator)
    out = num[:, :D] / num[:, D]

Round A (per l-half): one fp16 score matmul stream (64 q/k channels plus a
ones*(-m1) shift channel, fp32 PSUM accumulate), exp on ACT -> p (bf16,
kept in SBUF), and B = P @ V' accumulated on the PE (ones column gives Z).

Round B builds p5 = d^5 per 128-row s-tile via two engine paths, balancing
the ACT and DVE engines (the two all-stock bottlenecks):
  - DVE path (N_DVE tiles):  d = p*zb (zb = broadcast bf16 1/Z), then
    d2 = d*d, d4 = d2*d2, p5 = d4*d  -- four 2x-mode tensor_tensor ops.
  - ACT path (rest): recompute scores with one extra channel
    (-1 ones row) * (lnZ row written on-device by an ACT Ln, fp16), then
    p5 = exp(5 * (s - m1 - lnZ)) in a single ACT pass.
Both produce the same d^5 scale, accumulated into A5 = P5 @ V'.

zb = broadcast bf16(1/Z) is exp(-lnZ) on ACT (a DVE reciprocal over
[1,1024] is an 8-pass iterative divide, ~7us) replicated across partitions
by a K=1 PE matmul against a ones row.

Epilogue: num = A5 + B*(1/Z) (two DVE tensor_tensor ops straight from
PSUM/SBUF), then per 128 rows: PE transpose, DVE reciprocal of the
denominator, ACT scaled copy, DMA out.

The (pair, l-half) units are software-pipelined depth 1, emitted as
  [B_act(i-1) | A(i) | Ln/zr/b_sb(i) | B_rest_mms(i-1) | zp(i) | B_epi(i-1)]
so each engine queue stays busy: the exp5s of unit i-1 sit ahead of unit
i's exps on ACT; the DVE chains of i-1 run during A(i)'s PE/ACT work and
are not queued behind mid(i)'s zb cast (which waits on late-A(i) PE work);
the zp replicate matmuls cover the PE's wait on the num add; and the
epilogue transposes allocate from the ps_acc ring (whose two slots are both
dead by then) instead of the score ring, which otherwise serializes the
next unit's score matmuls against this unit's epilogue.  Within A, PV-B for
s-tile st is emitted after scores+exp of st+1 so the PE never head-blocks
on an exp.  The last unit's round B is mostly-ACT (its DVE chains would
drain serially with no next unit to overlap).  PSUM: 2 score buffers
(4 banks) + 2 accumulators (4 banks) = 8 banks exactly; Bacc is freed
early via an ACT copy to SBUF (GPSIMD cannot read PSUM).

Numerics: rel err ~4.6e-3 on hardware (gate 2e-2).

Sharding: B*H = 32 (b,h) pairs, 4 per core across 8 cores, no cross-core
communication.  kernel() takes full inputs and returns the full output.
Q/K are pre-transposed on the host and DMA'd in [66, L] layout.
"""

import os
import sys
import numpy as np

for _p in ("/opt/trn_rl_repo",):
    if os.path.isdir(_p) and _p not in sys.path:
        sys.path.insert(0, _p)

from contextlib import ExitStack

import json as _json

import ml_dtypes

import concourse.bass as bass
import concourse.mybir as mybir
import concourse.tile as tile
import concourse.bass2jax as _bass2jax
import concourse.bass_utils as _bass_utils
from concourse.bass_utils import run_bass_kernel_spmd
from concourse.masks import make_identity

# ---------------------------------------------------------------------------
# Workaround: this container's walrus build rejects instructions carrying
# more than one sync wait ("Too many sync wait commands").  Tile's scheduler
# freely attaches 2-3 waits per instruction.  Rewrite the BIR JSON before
# compilation: excess waits are hoisted onto freshly inserted same-engine
# NoOp instructions placed immediately before the instruction, one wait
# each.  Semantics are unchanged (waits are conjunctive >= conditions and
# engine program order is preserved).
# ---------------------------------------------------------------------------

_MAX_WAITS = 1


def _split_waits_in_bir(bir_json: bytes) -> bytes:
    bir = _json.loads(bir_json)
    n_new = [0]

    def fix_block(bb):
        out = []
        for inst in bb["instructions"]:
            si = inst.get("sync_info") or {}
            waits = si.get("on_wait") or []
            if len(waits) > _MAX_WAITS:
                excess, keep = waits[:-_MAX_WAITS], waits[-_MAX_WAITS:]
                for w in excess:
                    n_new[0] += 1
                    out.append({
                        "debug": inst.get("debug", 0),
                        "engine": inst["engine"],
                        "ins": [],
                        "name": "I-wsplit-%d" % n_new[0],
                        "opcode": "NoOp",
                        "outs": [],
                        "sync_info": {"on_update": [], "on_wait": [w]},
                    })
                si["on_wait"] = keep
            out.append(inst)
        bb["instructions"] = out

    for fn in bir["functions"]:
        for bb in fn["blocks"]:
            fix_block(bb)
    return _json.dumps(bir).encode()


_orig_compile_bir_kernel = _bass_utils.compile_bir_kernel


def _patched_compile_bir_kernel(bir_json, tmpdir, neff_name="file.neff"):
    return _orig_compile_bir_kernel(
        _split_waits_in_bir(bir_json), tmpdir, neff_name=neff_name
    )


_bass_utils.compile_bir_kernel = _patched_compile_bir_kernel
_bass2jax.compile_bir_kernel = _patched_compile_bir_kernel

# (walrus's --enable-ldw-opt dedup is incompatible with the framework's
# explicit InstLdweights preamble on this build; leave it off.)

# ---------------------------------------------------------------------------

B, L, S, H, E, D = 2, 2048, 2048, 16, 64, 64
NCORES = 8
NP = (B * H) // NCORES  # pairs per core = 4
EC = E + 2   # channels: 64 fp16 q/k + ones*(-m1) + (-1)*lnZ
DV = D + 1   # v columns: 64 data + ones column (carries Z / denominator)
LHALF = 1024
NCH = 2      # 512-wide matmul chunks (PSUM bank limit)
NLH = L // LHALF
ST = S // 128
N_DVE = 10   # s-tiles per l-half whose p5 is computed on the DVE
FACTOR = 5.0

F32 = mybir.dt.float32
BF16 = mybir.dt.bfloat16
FP16 = mybir.dt.float16
EXP = mybir.ActivationFunctionType.Exp
LN = mybir.ActivationFunctionType.Ln
COPY = mybir.ActivationFunctionType.Copy
MULT = mybir.AluOpType.mult
ADD = mybir.AluOpType.add

M_COEF = float(np.sqrt(2.0 * np.log(S)))
M_MARGIN = 25.0


def _emit(ctx: ExitStack, tc: tile.TileContext, qtd, ktd, vad, outp):
    nc = tc.nc

    const = ctx.enter_context(tc.tile_pool(name="const", bufs=1))
    kq = ctx.enter_context(tc.tile_pool(name="kq", bufs=2))
    vpool = ctx.enter_context(tc.tile_pool(name="vp", bufs=2))
    ppool = ctx.enter_context(tc.tile_pool(name="pp", bufs=2))
    p5pool = ctx.enter_context(tc.tile_pool(name="p5", bufs=4))
    dpool = ctx.enter_context(tc.tile_pool(name="dp", bufs=3))
    zpool = ctx.enter_context(tc.tile_pool(name="zp", bufs=2))
    eppool = ctx.enter_context(tc.tile_pool(name="ep", bufs=3))
    opool = ctx.enter_context(tc.tile_pool(name="op", bufs=4))

    ps_sc = ctx.enter_context(tc.tile_pool(name="ps_sc", bufs=2, space="PSUM"))
    ps_acc = ctx.enter_context(tc.tile_pool(name="ps_acc", bufs=2, space="PSUM"))

    ident65 = const.tile([DV, DV], F32)
    make_identity(nc, ident65)
    identb65 = const.tile([DV, DV], BF16)
    make_identity(nc, identb65)
    ones_row = const.tile([1, 128], BF16)
    nc.vector.memset(ones_row, 1.0)

    # per-pair tiles, loaded once (ring 2 across pairs)
    loaded = {}

    def load_pair(bh):
        qtT = kq.tile([EC, L], FP16, tag="qt", name="qtT")
        nc.sync.dma_start(out=qtT, in_=qtd[bh])
        ktT = kq.tile([EC, S], FP16, tag="kt", name="ktT")
        nc.sync.dma_start(out=ktT, in_=ktd[bh])
        vts = []
        for t in range(ST):
            vt = vpool.tile([128, DV], BF16, tag=f"v{t}", name=f"vt{t}")
            nc.sync.dma_start(out=vt, in_=vad[bh, t * 128:(t + 1) * 128, :])
            vts.append(vt)
        loaded[bh] = (qtT, ktT, vts)

    def emit_A(bh, lh):
        qtT, ktT, vts = loaded[bh]
        l0 = lh * LHALF
        bacc = ps_acc.tile([DV, LHALF], F32, tag="acc", name="bacc")
        ps = []

        def emit_pvb(st):
            for c in range(NCH):
                cs = slice(c * 512, (c + 1) * 512)
                nc.tensor.matmul(bacc[:, cs], lhsT=vts[st], rhs=ps[st][:, cs],
                                 start=(st == 0), stop=(st == ST - 1))

        # PV-B for s-tile st is emitted after the scores+exp of st+1, so the
        # PE never head-blocks on the exp it just requested
        for st in range(ST):
            sb = slice(st * 128, (st + 1) * 128)
            sc = ps_sc.tile([128, LHALF], F32, tag="sc", name="scA")
            for c in range(NCH):
                cs = slice(c * 512, (c + 1) * 512)
                gs = slice(l0 + c * 512, l0 + (c + 1) * 512)
                nc.tensor.matmul(sc[:, cs], lhsT=ktT[0:E + 1, sb],
                                 rhs=qtT[0:E + 1, gs], start=True, stop=True)
            p = ppool.tile([128, LHALF], BF16, tag=f"p{st}", name="p")
            nc.scalar.activation(p, sc, EXP, bias=0.0, scale=1.0)
            ps.append(p)
            if st > 0:
                emit_pvb(st - 1)
        emit_pvb(ST - 1)

        # lnZ channel (fp16 row 65 of qt, matched with -1 ones row in kt).
        # Engine writes must start at a 32-aligned partition, so Ln lands in
        # an aligned scratch row and a tiny SBUF->SBUF DMA moves it into place.
        lz = zpool.tile([1, LHALF], FP16, tag="lz", name="lz")
        nc.scalar.activation(lz, bacc[D:DV, :], LN, bias=0.0, scale=1.0)
        nc.sync.dma_start(out=qtT[E + 1:E + 2, l0:l0 + LHALF], in_=lz)
        # zb = broadcast bf16(1/Z); 1/Z = exp(-lnZ) on ACT (a DVE reciprocal
        # over [1,1024] is an 8-pass iterative divide, ~7us -- far slower)
        zr = zpool.tile([1, LHALF], BF16, tag="zr", name="zr")
        nc.scalar.activation(zr, lz, EXP, bias=0.0, scale=-1.0)
        zp = ps_sc.tile([128, LHALF], F32, tag="sc", name="zp")
        for c in range(NCH):
            cs = slice(c * 512, (c + 1) * 512)
            nc.tensor.matmul(zp[:, cs], lhsT=ones_row, rhs=zr[:, cs],
                             start=True, stop=True)
        zb = zpool.tile([128, LHALF], BF16, tag="zb", name="zb")
        nc.vector.tensor_copy(zb, zp)
        # free bacc early (GPSIMD cannot read PSUM, so ACT does the copy)
        b_sb = eppool.tile([DV, LHALF], F32, tag="bsb", name="b_sb")
        nc.scalar.activation(b_sb, bacc, COPY, bias=0.0, scale=1.0)
        return {"ps": ps, "zb": zb, "b_sb": b_sb, "lh": lh, "bh": bh}

    N_ACT = ST - N_DVE

    def emit_B_act(stA):
        """Round-B ACT-path tiles (sts 0..n_act-1): score recompute with the
        lnZ channel + exp(5*) on ACT.  Emitted BEFORE the next unit's A-phase
        so the exp5s sit early in the ACT queue."""
        bh, lh = stA["bh"], stA["lh"]
        qtT, ktT, vts = loaded[bh]
        l0 = lh * LHALF
        a5 = ps_acc.tile([DV, LHALF], F32, tag="acc", name="a5")
        stA["a5"] = a5
        # t1 = B/Z only needs b_sb and zb (ready since this unit's mid);
        # computing it here keeps the epilogue's num = t1 + A5 one DVE op
        # after the last chain instead of two
        t1 = eppool.tile([DV, LHALF], F32, tag="t1", name="t1")
        nc.vector.tensor_tensor(out=t1, in0=stA["b_sb"], in1=stA["zb"][0:DV, :],
                                op=MULT)
        stA["t1"] = t1
        p5s = {}

        def emit_a5(st):
            for c in range(NCH):
                cs = slice(c * 512, (c + 1) * 512)
                nc.tensor.matmul(a5[:, cs], lhsT=vts[st], rhs=p5s[st][:, cs],
                                 start=(st == 0), stop=(st == ST - 1))

        for st in range(stA["n_act"]):
            sb = slice(st * 128, (st + 1) * 128)
            sc = ps_sc.tile([128, LHALF], F32, tag="sc", name="scB")
            for c in range(NCH):
                cs = slice(c * 512, (c + 1) * 512)
                gs = slice(l0 + c * 512, l0 + (c + 1) * 512)
                nc.tensor.matmul(sc[:, cs], lhsT=ktT[:, sb],
                                 rhs=qtT[:, gs], start=True, stop=True)
            p5 = p5pool.tile([128, LHALF], BF16, tag="p5a", name="p5")
            nc.scalar.activation(p5, sc, EXP, bias=0.0, scale=FACTOR)
            p5s[st] = p5
            if st > 0:
                emit_a5(st - 1)
        emit_a5(stA["n_act"] - 1)

    def emit_B_rest(stA):
        """Round-B DVE-path tiles (sts N_ACT..15) + combine + epilogue."""
        bh, lh = stA["bh"], stA["lh"]
        qtT, ktT, vts = loaded[bh]
        ps, zb, b_sb, a5 = stA["ps"], stA["zb"], stA["b_sb"], stA["a5"]
        l0 = lh * LHALF
        for st in range(stA["n_act"], ST):
            d = dpool.tile([128, LHALF], BF16, tag="d", name="d")
            nc.vector.tensor_tensor(out=d, in0=ps[st], in1=zb, op=MULT)
            d2 = dpool.tile([128, LHALF], BF16, tag="d2", name="d2")
            nc.vector.tensor_tensor(out=d2, in0=d, in1=d, op=MULT)
            d4 = dpool.tile([128, LHALF], BF16, tag="d4", name="d4")
            nc.vector.tensor_tensor(out=d4, in0=d2, in1=d2, op=MULT)
            p5 = p5pool.tile([128, LHALF], BF16, tag="p5", name="p5")
            nc.vector.tensor_tensor(out=p5, in0=d4, in1=d, op=MULT)
            for c in range(NCH):
                cs = slice(c * 512, (c + 1) * 512)
                nc.tensor.matmul(a5[:, cs], lhsT=vts[st], rhs=p5[:, cs],
                                 start=False, stop=(st == ST - 1))

        # combine: num = A5 + B/Z (one DVE op; t1 was computed in B_act)
        t1 = stA["t1"]
        num = eppool.tile([DV, LHALF], F32, tag="num", name="num")
        nc.vector.tensor_tensor(out=num, in0=t1, in1=a5, op=ADD)
        stA["num"] = num

    def emit_B_epi(stA):
        """out = num[:, :D] / num[:, D].  The nt transposes allocate from the
        ps_acc ring (both of its slots are dead here: bacc freed by the early
        b_sb copy, a5 by the num add) so the sc ring is left free for the
        next unit's score tiles."""
        bh, lh, num = stA["bh"], stA["lh"], stA["num"]
        l0 = lh * LHALF
        for ch in range(LHALF // 128):
            nt = ps_acc.tile([128, DV], F32, tag="acc", name="nt")
            nc.tensor.transpose(nt, num[:, ch * 128:(ch + 1) * 128], ident65)
            rz = opool.tile([128, 1], F32, tag="rz", name="rz")
            nc.vector.reciprocal(rz, nt[:, D:DV])
            ot = opool.tile([128, D], F32, tag="ot", name="ot")
            nc.scalar.activation(ot, nt[:, 0:D], COPY, bias=0.0,
                                 scale=rz[:, 0:1])
            lrow = l0 + ch * 128
            nc.gpsimd.dma_start(out=outp[bh, lrow:lrow + 128, :], in_=ot)

    units = [(bh, lh) for bh in range(NP) for lh in range(NLH)]
    pending = None
    for i, (bh, lh) in enumerate(units):
        if lh == 0:
            load_pair(bh)
        if pending is not None:
            emit_B_act(pending)
        qtT, ktT, vts = loaded[bh]
        l0 = lh * LHALF
        bacc = ps_acc.tile([DV, LHALF], F32, tag="acc", name="bacc")
        ps = []

        def emit_pvb(st):
            for c in range(NCH):
                cs = slice(c * 512, (c + 1) * 512)
                nc.tensor.matmul(bacc[:, cs], lhsT=vts[st], rhs=ps[st][:, cs],
                                 start=(st == 0), stop=(st == ST - 1))

        # PV-B for s-tile st is emitted after the scores+exp of st+1, so the
        # PE never head-blocks on the exp it just requested
        for st in range(ST):
            sb = slice(st * 128, (st + 1) * 128)
            sc = ps_sc.tile([128, LHALF], F32, tag="sc", name="scA")
            for c in range(NCH):
                cs = slice(c * 512, (c + 1) * 512)
                gs = slice(l0 + c * 512, l0 + (c + 1) * 512)
                nc.tensor.matmul(sc[:, cs], lhsT=ktT[0:E + 1, sb],
                                 rhs=qtT[0:E + 1, gs], start=True, stop=True)
            p = ppool.tile([128, LHALF], BF16, tag=f"p{st}", name="p")
            nc.scalar.activation(p, sc, EXP, bias=0.0, scale=1.0)
            ps.append(p)
            if st > 0:
                emit_pvb(st - 1)
        emit_pvb(ST - 1)

        # lnZ chain early: Ln gates the lnZ DMA and thus every B_act(i) PE
        # matmul; emitting it here puts it ahead of the previous unit's
        # epilogue copies in the ACT queue.
        lz = zpool.tile([1, LHALF], FP16, tag="lz", name="lz")
        nc.scalar.activation(lz, bacc[D:DV, :], LN, bias=0.0, scale=1.0)
        nc.sync.dma_start(out=qtT[E + 1:E + 2, l0:l0 + LHALF], in_=lz)
        zr = zpool.tile([1, LHALF], BF16, tag="zr", name="zr")
        nc.scalar.activation(zr, lz, EXP, bias=0.0, scale=-1.0)
        # free bacc now: as the LAST ACT op of a cycle this stalled both the
        # next a5 ring allocation (PE) and the hoisted t1 (DVE head-of-line)
        b_sb = eppool.tile([DV, LHALF], F32, tag="bsb", name="b_sb")
        nc.scalar.activation(b_sb, bacc, COPY, bias=0.0, scale=1.0)

        # B_rest(i-1): its DVE chains are ready (zb of i-1 exists) and must
        # not queue behind mid(i)'s zb cast, which waits on late PE work.
        if pending is not None:
            emit_B_rest(pending)

        # ---- mid: zb broadcast.  The zp matmuls sit between B_rest(i-1)'s
        # A5 matmuls and its epilogue transposes in the PE queue, covering
        # the PE's wait on the num add (DVE).
        zp = ps_sc.tile([128, LHALF], F32, tag="sc", name="zp")
        for c in range(NCH):
            cs = slice(c * 512, (c + 1) * 512)
            nc.tensor.matmul(zp[:, cs], lhsT=ones_row, rhs=zr[:, cs],
                             start=True, stop=True)
        zb = zpool.tile([128, LHALF], BF16, tag="zb", name="zb")
        nc.vector.tensor_copy(zb, zp)

        if pending is not None:
            emit_B_epi(pending)

        # the last unit's round B drains with no A-phase to overlap; give it
        # all-ACT tiles so its p5 work overlaps its own PE/ACT instead of
        # serializing on the DVE chain
        # last unit: no next unit overlaps its round B, so balance its p5
        # work across ACT (exp5) and DVE (chains) instead of one engine
        n_act = N_ACT if i < len(units) - 1 else 10
        pending = {"ps": ps, "zb": zb, "b_sb": b_sb, "lh": lh, "bh": bh,
                   "n_act": n_act}
    emit_B_act(pending)
    emit_B_rest(pending)
    emit_B_epi(pending)


_CACHE = {}


def _build():
    if "nc" in _CACHE:
        return _CACHE["nc"]
    nc = bass.Bass()
    qtd = nc.declare_dram_parameter("qt", [NP, EC, L], FP16, isOutput=False)
    ktd = nc.declare_dram_parameter("kt", [NP, EC, S], FP16, isOutput=False)
    vad = nc.declare_dram_parameter("va", [NP, S, DV], BF16, isOutput=False)
    outp = nc.declare_dram_parameter("out", [NP, L, D], F32, isOutput=True)
    with tile.TileContext(nc) as tc:
        with ExitStack() as ctx:
            _emit(ctx, tc, qtd[:], ktd[:], vad[:], outp[:])
    _CACHE["nc"] = nc
    return nc


def _prep_inputs(queries, keys, values):
    bf = ml_dtypes.bfloat16
    q = np.ascontiguousarray(np.asarray(queries, np.float32)
                             .transpose(0, 2, 1, 3)).reshape(B * H, L, E)
    k = np.ascontiguousarray(np.asarray(keys, np.float32)
                             .transpose(0, 2, 1, 3)).reshape(B * H, S, E)
    v = np.ascontiguousarray(np.asarray(values, np.float32)
                             .transpose(0, 2, 1, 3)).reshape(B * H, S, D)
    m1 = (M_COEF * np.sqrt((q.astype(np.float64) ** 2).sum(-1)) + M_MARGIN
          ).astype(np.float32)  # [BH, L]
    one_s = np.ones((B * H, S, 1), np.float32)
    zero_l = np.zeros((B * H, L, 1), np.float32)
    # qt channels: q | -m1 | lnZ placeholder (written on device per l-half)
    qt = np.concatenate([q, -m1[..., None], zero_l], axis=-1)   # [., L, 66]
    # kt channels: k | +1 (pairs with -m1) | -1 (pairs with lnZ)
    kt = np.concatenate([k, one_s, -one_s], axis=-1)            # [., S, 66]
    qt = np.ascontiguousarray(qt.transpose(0, 2, 1)).astype(np.float16)
    kt = np.ascontiguousarray(kt.transpose(0, 2, 1)).astype(np.float16)
    va = np.concatenate([v.astype(bf), one_s.astype(bf)], axis=-1)
    in_maps = []
    for c in range(NCORES):
        sl = slice(c * NP, (c + 1) * NP)
        in_maps.append({
            "qt": np.ascontiguousarray(qt[sl]),
            "kt": np.ascontiguousarray(kt[sl]),
            "va": np.ascontiguousarray(va[sl]),
        })
    return in_maps


def _gather(results):
    outs = np.stack([results[c]["out"] for c in range(NCORES)])  # [8,NP,L,D]
    out = outs.reshape(B, H, L, D).transpose(0, 2, 1, 3)
    return np.ascontiguousarray(out)


def run_sharded(queries, keys, values, **kw):
    """Run on the 8 neuron cores; returns (full_output, BassKernelResults)."""
    nc = _build()
    in_maps = _prep_inputs(queries, keys, values)
    res = run_bass_kernel_spmd(nc, in_maps, list(range(NCORES)), **kw)
    return _gather(res.results), res


def kernel(queries, keys, values):
    out, _ = run_sharded(queries, keys, values)
    return out


# revision 38
# speedup vs baseline: 1.0146x; 1.0051x over previous
"""Trainium2 Bass kernel for DynamicSparseAttention.

Reference computation (per batch b, head h):
    scores  = Q @ K^T                      [L, S]
    dense   = softmax(scores, axis=-1)
    routing = dense ** 5
    combined = (routing + dense) * 0.5
    sparse  = combined / sum(combined, -1, keepdims=True)
    out     = sparse @ V                   [L, D]

Math: let p = exp(s - m1) with a per-row analytic upper bound m1 and
Z = sum_s p.  d = p/Z is the exact softmax, and with V' = [V | 1]:
    num = D5 @ V' + (P @ V')/Z   (rows 0..63 numerator, row 64 denominator)
    out = num[:, :D] / num[:, D]

Round A (per l-half): one fp16 score matmul stream (64 q/k channels plus a
ones*(-m1) shift channel, fp32 PSUM accumulate), exp on ACT -> p (bf16,
kept in SBUF), and B = P @ V' accumulated on the PE (ones column gives Z).

Round B builds p5 = d^5 per 128-row s-tile via two engine paths, balancing
the ACT and DVE engines (the two all-stock bottlenecks):
  - DVE path (N_DVE tiles):  d = p*zb (zb = broadcast bf16 1/Z), then
    d2 = d*d, d4 = d2*d2, p5 = d4*d  -- four 2x-mode tensor_tensor ops.
  - ACT path (rest): recompute scores with one extra channel
    (-1 ones row) * (lnZ row written on-device by an ACT Ln, fp16), then
    p5 = exp(5 * (s - m1 - lnZ)) in a single ACT pass.
Both produce the same d^5 scale, accumulated into A5 = P5 @ V'.

zb = broadcast bf16(1/Z) is exp(-lnZ) on ACT (a DVE reciprocal over
[1,1024] is an 8-pass iterative divide, ~7us) replicated across partitions
by a K=1 PE matmul against a ones row.

Epilogue: num = A5 + B*(1/Z) (two DVE tensor_tensor ops straight from
PSUM/SBUF), then per 128 rows: PE transpose, DVE reciprocal of the
denominator, ACT scaled copy, DMA out.

The (pair, l-half) units are software-pipelined depth 1, emitted as
  [B_act(i-1) | A(i) | Ln/zr/b_sb(i) | B_rest_mms(i-1) | zp(i) | B_epi(i-1)]
so each engine queue stays busy: the exp5s of unit i-1 sit ahead of unit
i's exps on ACT; the DVE chains of i-1 run during A(i)'s PE/ACT work and
are not queued behind mid(i)'s zb cast (which waits on late-A(i) PE work);
the zp replicate matmuls cover the PE's wait on the num add; and the
epilogue transposes allocate from the ps_acc ring (whose two slots are both
dead by then) instead of the score ring, which otherwise serializes the
next unit's score matmuls against this unit's epilogue.  Within A, PV-B for
s-tile st is emitted after scores+exp of st+1 so the PE never head-blocks
on an exp.  The last unit's round B is mostly-ACT (its DVE chains would
drain serially with no next unit to overlap).  PSUM: 2 score buffers
(4 banks) + 2 accumulators (4 banks) = 8 banks exactly; Bacc is freed
early via an ACT copy to SBUF (GPSIMD cannot read PSUM).

Numerics: rel err ~4.6e-3 on hardware (gate 2e-2).

Sharding: B*H = 32 (b,h) pairs, 4 per core across 8 cores, no cross-core
communication.  kernel() takes full inputs and returns the full output.
Q/K are pre-transposed on the host and DMA'd in [66, L] layout.
"""

import os
import sys
import numpy as np

for _p in ("/opt/trn_rl_repo",):
    if os.path.isdir(_p) and _p not in sys.path:
        sys.path.insert(0, _p)

from contextlib import ExitStack

import json as _json

import ml_dtypes

import concourse.bass as bass
import concourse.mybir as mybir
import concourse.tile as tile
import concourse.bass2jax as _bass2jax
import concourse.bass_utils as _bass_utils
from concourse.bass_utils import run_bass_kernel_spmd
from concourse.masks import make_identity

# ---------------------------------------------------------------------------
# Workaround: this container's walrus build rejects instructions carrying
# more than one sync wait ("Too many sync wait commands").  Tile's scheduler
# freely attaches 2-3 waits per instruction.  Rewrite the BIR JSON before
# compilation: excess waits are hoisted onto freshly inserted same-engine
# NoOp instructions placed immediately before the instruction, one wait
# each.  Semantics are unchanged (waits are conjunctive >= conditions and
# engine program order is preserved).
# ---------------------------------------------------------------------------

_MAX_WAITS = 1


def _split_waits_in_bir(bir_json: bytes) -> bytes:
    bir = _json.loads(bir_json)
    n_new = [0]

    def fix_block(bb):
        out = []
        for inst in bb["instructions"]:
            si = inst.get("sync_info") or {}
            waits = si.get("on_wait") or []
            if len(waits) > _MAX_WAITS:
                excess, keep = waits[:-_MAX_WAITS], waits[-_MAX_WAITS:]
                for w in excess:
                    n_new[0] += 1
                    out.append({
                        "debug": inst.get("debug", 0),
                        "engine": inst["engine"],
                        "ins": [],
                        "name": "I-wsplit-%d" % n_new[0],
                        "opcode": "NoOp",
                        "outs": [],
                        "sync_info": {"on_update": [], "on_wait": [w]},
                    })
                si["on_wait"] = keep
            out.append(inst)
        bb["instructions"] = out

    for fn in bir["functions"]:
        for bb in fn["blocks"]:
            fix_block(bb)
    return _json.dumps(bir).encode()


_orig_compile_bir_kernel = _bass_utils.compile_bir_kernel


def _patched_compile_bir_kernel(bir_json, tmpdir, neff_name="file.neff"):
    return _orig_compile_bir_kernel(
        _split_waits_in_bir(bir_json), tmpdir, neff_name=neff_name
    )


_bass_utils.compile_bir_kernel = _patched_compile_bir_kernel
_bass2jax.compile_bir_kernel = _patched_compile_bir_kernel

# (walrus's --enable-ldw-opt dedup is incompatible with the framework's
# explicit InstLdweights preamble on this build; leave it off.)

# ---------------------------------------------------------------------------

B, L, S, H, E, D = 2, 2048, 2048, 16, 64, 64
NCORES = 8
NP = (B * H) // NCORES  # pairs per core = 4
EC = E + 2   # channels: 64 fp16 q/k + ones*(-m1) + (-1)*lnZ
DV = D + 1   # v columns: 64 data + ones column (carries Z / denominator)
LHALF = 1024
NCH = 2      # 512-wide matmul chunks (PSUM bank limit)
NLH = L // LHALF
ST = S // 128
N_DVE = 10   # s-tiles per l-half whose p5 is computed on the DVE
FACTOR = 5.0

F32 = mybir.dt.float32
BF16 = mybir.dt.bfloat16
FP16 = mybir.dt.float16
EXP = mybir.ActivationFunctionType.Exp
LN = mybir.ActivationFunctionType.Ln
COPY = mybir.ActivationFunctionType.Copy
MULT = mybir.AluOpType.mult
ADD = mybir.AluOpType.add

M_COEF = float(np.sqrt(2.0 * np.log(S)))
M_MARGIN = 25.0


def _emit(ctx: ExitStack, tc: tile.TileContext, qtd, ktd, vad, outp):
    nc = tc.nc

    const = ctx.enter_context(tc.tile_pool(name="const", bufs=1))
    kq = ctx.enter_context(tc.tile_pool(name="kq", bufs=2))
    vpool = ctx.enter_context(tc.tile_pool(name="vp", bufs=2))
    ppool = ctx.enter_context(tc.tile_pool(name="pp", bufs=2))
    p5pool = ctx.enter_context(tc.tile_pool(name="p5", bufs=4))
    dpool = ctx.enter_context(tc.tile_pool(name="dp", bufs=3))
    zpool = ctx.enter_context(tc.tile_pool(name="zp", bufs=2))
    eppool = ctx.enter_context(tc.tile_pool(name="ep", bufs=3))
    opool = ctx.enter_context(tc.tile_pool(name="op", bufs=8))

    ps_sc = ctx.enter_context(tc.tile_pool(name="ps_sc", bufs=2, space="PSUM"))
    ps_acc = ctx.enter_context(tc.tile_pool(name="ps_acc", bufs=2, space="PSUM"))

    ident65 = const.tile([DV, DV], F32)
    make_identity(nc, ident65)
    identb65 = const.tile([DV, DV], BF16)
    make_identity(nc, identb65)
    ones_row = const.tile([1, 128], BF16)
    nc.vector.memset(ones_row, 1.0)

    # per-pair tiles, loaded once (ring 2 across pairs)
    loaded = {}

    def load_pair(bh):
        qtT = kq.tile([EC, L], FP16, tag="qt", name="qtT")
        nc.sync.dma_start(out=qtT, in_=qtd[bh])
        ktT = kq.tile([EC, S], FP16, tag="kt", name="ktT")
        nc.sync.dma_start(out=ktT, in_=ktd[bh])
        vts = []
        for t in range(ST):
            vt = vpool.tile([128, DV], BF16, tag=f"v{t}", name=f"vt{t}")
            nc.sync.dma_start(out=vt, in_=vad[bh, t * 128:(t + 1) * 128, :])
            vts.append(vt)
        loaded[bh] = (qtT, ktT, vts)

    def emit_A(bh, lh):
        qtT, ktT, vts = loaded[bh]
        l0 = lh * LHALF
        bacc = ps_acc.tile([DV, LHALF], F32, tag="acc", name="bacc")
        ps = []

        def emit_pvb(st):
            for c in range(NCH):
                cs = slice(c * 512, (c + 1) * 512)
                nc.tensor.matmul(bacc[:, cs], lhsT=vts[st], rhs=ps[st][:, cs],
                                 start=(st == 0), stop=(st == ST - 1))

        # PV-B for s-tile st is emitted after the scores+exp of st+1, so the
        # PE never head-blocks on the exp it just requested
        for st in range(ST):
            sb = slice(st * 128, (st + 1) * 128)
            sc = ps_sc.tile([128, LHALF], F32, tag="sc", name="scA")
            for c in range(NCH):
                cs = slice(c * 512, (c + 1) * 512)
                gs = slice(l0 + c * 512, l0 + (c + 1) * 512)
                nc.tensor.matmul(sc[:, cs], lhsT=ktT[0:E + 1, sb],
                                 rhs=qtT[0:E + 1, gs], start=True, stop=True)
            p = ppool.tile([128, LHALF], BF16, tag=f"p{st}", name="p")
            nc.scalar.activation(p, sc, EXP, bias=0.0, scale=1.0)
            ps.append(p)
            if st > 0:
                emit_pvb(st - 1)
        emit_pvb(ST - 1)

        # lnZ channel (fp16 row 65 of qt, matched with -1 ones row in kt).
        # Engine writes must start at a 32-aligned partition, so Ln lands in
        # an aligned scratch row and a tiny SBUF->SBUF DMA moves it into place.
        lz = zpool.tile([1, LHALF], FP16, tag="lz", name="lz")
        nc.scalar.activation(lz, bacc[D:DV, :], LN, bias=0.0, scale=1.0)
        nc.sync.dma_start(out=qtT[E + 1:E + 2, l0:l0 + LHALF], in_=lz)
        # zb = broadcast bf16(1/Z); 1/Z = exp(-lnZ) on ACT (a DVE reciprocal
        # over [1,1024] is an 8-pass iterative divide, ~7us -- far slower)
        zr = zpool.tile([1, LHALF], BF16, tag="zr", name="zr")
        nc.scalar.activation(zr, lz, EXP, bias=0.0, scale=-1.0)
        zp = ps_sc.tile([128, LHALF], F32, tag="sc", name="zp")
        for c in range(NCH):
            cs = slice(c * 512, (c + 1) * 512)
            nc.tensor.matmul(zp[:, cs], lhsT=ones_row, rhs=zr[:, cs],
                             start=True, stop=True)
        zb = zpool.tile([128, LHALF], BF16, tag="zb", name="zb")
        nc.vector.tensor_copy(zb, zp)
        # free bacc early (GPSIMD cannot read PSUM, so ACT does the copy)
        b_sb = eppool.tile([DV, LHALF], F32, tag="bsb", name="b_sb")
        nc.scalar.activation(b_sb, bacc, COPY, bias=0.0, scale=1.0)
        return {"ps": ps, "zb": zb, "b_sb": b_sb, "lh": lh, "bh": bh}

    N_ACT = ST - N_DVE

    def emit_B_act(stA):
        """Round-B ACT-path tiles (sts 0..n_act-1): score recompute with the
        lnZ channel + exp(5*) on ACT.  Emitted BEFORE the next unit's A-phase
        so the exp5s sit early in the ACT queue."""
        bh, lh = stA["bh"], stA["lh"]
        qtT, ktT, vts = loaded[bh]
        l0 = lh * LHALF
        a5 = ps_acc.tile([DV, LHALF], F32, tag="acc", name="a5")
        stA["a5"] = a5
        # t1 = B/Z only needs b_sb and zb (ready since this unit's mid);
        # computing it here keeps the epilogue's num = t1 + A5 one DVE op
        # after the last chain instead of two
        t1 = eppool.tile([DV, LHALF], F32, tag="t1", name="t1")
        nc.vector.tensor_tensor(out=t1, in0=stA["b_sb"], in1=stA["zb"][0:DV, :],
                                op=MULT)
        stA["t1"] = t1
        p5s = {}

        def emit_a5(st):
            for c in range(NCH):
                cs = slice(c * 512, (c + 1) * 512)
                nc.tensor.matmul(a5[:, cs], lhsT=vts[st], rhs=p5s[st][:, cs],
                                 start=(st == 0), stop=(st == ST - 1))

        for st in range(stA["n_act"]):
            sb = slice(st * 128, (st + 1) * 128)
            sc = ps_sc.tile([128, LHALF], F32, tag="sc", name="scB")
            for c in range(NCH):
                cs = slice(c * 512, (c + 1) * 512)
                gs = slice(l0 + c * 512, l0 + (c + 1) * 512)
                nc.tensor.matmul(sc[:, cs], lhsT=ktT[:, sb],
                                 rhs=qtT[:, gs], start=True, stop=True)
            p5 = p5pool.tile([128, LHALF], BF16, tag="p5a", name="p5")
            nc.scalar.activation(p5, sc, EXP, bias=0.0, scale=FACTOR)
            p5s[st] = p5
            if st > 0:
                emit_a5(st - 1)
        emit_a5(stA["n_act"] - 1)

    def emit_B_rest(stA):
        """Round-B DVE-path tiles (sts N_ACT..15) + combine + epilogue."""
        bh, lh = stA["bh"], stA["lh"]
        qtT, ktT, vts = loaded[bh]
        ps, zb, b_sb, a5 = stA["ps"], stA["zb"], stA["b_sb"], stA["a5"]
        l0 = lh * LHALF
        for st in range(stA["n_act"], ST):
            d = dpool.tile([128, LHALF], BF16, tag="d", name="d")
            nc.vector.tensor_tensor(out=d, in0=ps[st], in1=zb, op=MULT)
            d2 = dpool.tile([128, LHALF], BF16, tag="d2", name="d2")
            nc.vector.tensor_tensor(out=d2, in0=d, in1=d, op=MULT)
            d4 = dpool.tile([128, LHALF], BF16, tag="d4", name="d4")
            nc.vector.tensor_tensor(out=d4, in0=d2, in1=d2, op=MULT)
            p5 = p5pool.tile([128, LHALF], BF16, tag="p5", name="p5")
            nc.vector.tensor_tensor(out=p5, in0=d4, in1=d, op=MULT)
            for c in range(NCH):
                cs = slice(c * 512, (c + 1) * 512)
                nc.tensor.matmul(a5[:, cs], lhsT=vts[st], rhs=p5[:, cs],
                                 start=False, stop=(st == ST - 1))

        # combine: num = A5 + B/Z (one DVE op; t1 was computed in B_act)
        t1 = stA["t1"]
        num = eppool.tile([DV, LHALF], F32, tag="num", name="num")
        nc.vector.tensor_tensor(out=num, in0=t1, in1=a5, op=ADD)
        stA["num"] = num

    def emit_B_epi(stA):
        """out = num[:, :D] / num[:, D].  The nt transposes allocate from the
        ps_acc ring (both of its slots are dead here: bacc freed by the early
        b_sb copy, a5 by the num add) so the sc ring is left free for the
        next unit's score tiles."""
        bh, lh, num = stA["bh"], stA["lh"], stA["num"]
        l0 = lh * LHALF
        for ch in range(LHALF // 128):
            nt = ps_acc.tile([128, DV], F32, tag="acc", name="nt")
            nc.tensor.transpose(nt, num[:, ch * 128:(ch + 1) * 128], ident65)
            rz = opool.tile([128, 1], F32, tag="rz", name="rz")
            nc.vector.reciprocal(rz, nt[:, D:DV])
            ot = opool.tile([128, D], F32, tag="ot", name="ot")
            nc.scalar.activation(ot, nt[:, 0:D], COPY, bias=0.0,
                                 scale=rz[:, 0:1])
            lrow = l0 + ch * 128
            nc.gpsimd.dma_start(out=outp[bh, lrow:lrow + 128, :], in_=ot)

    units = [(bh, lh) for bh in range(NP) for lh in range(NLH)]
    pending = None
    for i, (bh, lh) in enumerate(units):
        if lh == 0:
            load_pair(bh)
        if pending is not None:
            emit_B_act(pending)
        qtT, ktT, vts = loaded[bh]
        l0 = lh * LHALF
        bacc = ps_acc.tile([DV, LHALF], F32, tag="acc", name="bacc")
        ps = []

        def emit_pvb(st):
            for c in range(NCH):
                cs = slice(c * 512, (c + 1) * 512)
                nc.tensor.matmul(bacc[:, cs], lhsT=vts[st], rhs=ps[st][:, cs],
                                 start=(st == 0), stop=(st == ST - 1))

        # PV-B for s-tile st is emitted after the scores+exp of st+1, so the
        # PE never head-blocks on the exp it just requested
        for st in range(ST):
            sb = slice(st * 128, (st + 1) * 128)
            sc = ps_sc.tile([128, LHALF], F32, tag="sc", name="scA")
            for c in range(NCH):
                cs = slice(c * 512, (c + 1) * 512)
                gs = slice(l0 + c * 512, l0 + (c + 1) * 512)
                nc.tensor.matmul(sc[:, cs], lhsT=ktT[0:E + 1, sb],
                                 rhs=qtT[0:E + 1, gs], start=True, stop=True)
            p = ppool.tile([128, LHALF], BF16, tag=f"p{st}", name="p")
            nc.scalar.activation(p, sc, EXP, bias=0.0, scale=1.0)
            ps.append(p)
            if st > 0:
                emit_pvb(st - 1)
        emit_pvb(ST - 1)

        # lnZ chain early: Ln gates the lnZ DMA and thus every B_act(i) PE
        # matmul; emitting it here puts it ahead of the previous unit's
        # epilogue copies in the ACT queue.
        lz = zpool.tile([1, LHALF], FP16, tag="lz", name="lz")
        nc.scalar.activation(lz, bacc[D:DV, :], LN, bias=0.0, scale=1.0)
        nc.sync.dma_start(out=qtT[E + 1:E + 2, l0:l0 + LHALF], in_=lz)
        zr = zpool.tile([1, LHALF], BF16, tag="zr", name="zr")
        nc.scalar.activation(zr, lz, EXP, bias=0.0, scale=-1.0)
        # free bacc now: as the LAST ACT op of a cycle this stalled both the
        # next a5 ring allocation (PE) and the hoisted t1 (DVE head-of-line)
        b_sb = eppool.tile([DV, LHALF], F32, tag="bsb", name="b_sb")
        nc.scalar.activation(b_sb, bacc, COPY, bias=0.0, scale=1.0)

        # B_rest(i-1): its DVE chains are ready (zb of i-1 exists) and must
        # not queue behind mid(i)'s zb cast, which waits on late PE work.
        if pending is not None:
            emit_B_rest(pending)

        # ---- mid: zb broadcast.  The zp matmuls sit between B_rest(i-1)'s
        # A5 matmuls and its epilogue transposes in the PE queue, covering
        # the PE's wait on the num add (DVE).
        zp = ps_sc.tile([128, LHALF], F32, tag="sc", name="zp")
        for c in range(NCH):
            cs = slice(c * 512, (c + 1) * 512)
            nc.tensor.matmul(zp[:, cs], lhsT=ones_row, rhs=zr[:, cs],
                             start=True, stop=True)
        zb = zpool.tile([128, LHALF], BF16, tag="zb", name="zb")
        nc.vector.tensor_copy(zb, zp)

        if pending is not None:
            emit_B_epi(pending)

        # the last unit's round B drains with no A-phase to overlap; give it
        # all-ACT tiles so its p5 work overlaps its own PE/ACT instead of
        # serializing on the DVE chain
        # last unit: no next unit overlaps its round B, so balance its p5
        # work across ACT (exp5) and DVE (chains) instead of one engine
        n_act = N_ACT if i < len(units) - 1 else 10
        pending = {"ps": ps, "zb": zb, "b_sb": b_sb, "lh": lh, "bh": bh,
                   "n_act": n_act}
    emit_B_act(pending)
    emit_B_rest(pending)
    emit_B_epi(pending)


_CACHE = {}


def _build():
    if "nc" in _CACHE:
        return _CACHE["nc"]
    nc = bass.Bass()
    qtd = nc.declare_dram_parameter("qt", [NP, EC, L], FP16, isOutput=False)
    ktd = nc.declare_dram_parameter("kt", [NP, EC, S], FP16, isOutput=False)
    vad = nc.declare_dram_parameter("va", [NP, S, DV], BF16, isOutput=False)
    outp = nc.declare_dram_parameter("out", [NP, L, D], F32, isOutput=True)
    with tile.TileContext(nc) as tc:
        with ExitStack() as ctx:
            _emit(ctx, tc, qtd[:], ktd[:], vad[:], outp[:])
    _CACHE["nc"] = nc
    return nc


def _prep_inputs(queries, keys, values):
    bf = ml_dtypes.bfloat16
    q = np.ascontiguousarray(np.asarray(queries, np.float32)
                             .transpose(0, 2, 1, 3)).reshape(B * H, L, E)
    k = np.ascontiguousarray(np.asarray(keys, np.float32)
                             .transpose(0, 2, 1, 3)).reshape(B * H, S, E)
    v = np.ascontiguousarray(np.asarray(values, np.float32)
                             .transpose(0, 2, 1, 3)).reshape(B * H, S, D)
    m1 = (M_COEF * np.sqrt((q.astype(np.float64) ** 2).sum(-1)) + M_MARGIN
          ).astype(np.float32)  # [BH, L]
    one_s = np.ones((B * H, S, 1), np.float32)
    zero_l = np.zeros((B * H, L, 1), np.float32)
    # qt channels: q | -m1 | lnZ placeholder (written on device per l-half)
    qt = np.concatenate([q, -m1[..., None], zero_l], axis=-1)   # [., L, 66]
    # kt channels: k | +1 (pairs with -m1) | -1 (pairs with lnZ)
    kt = np.concatenate([k, one_s, -one_s], axis=-1)            # [., S, 66]
    qt = np.ascontiguousarray(qt.transpose(0, 2, 1)).astype(np.float16)
    kt = np.ascontiguousarray(kt.transpose(0, 2, 1)).astype(np.float16)
    va = np.concatenate([v.astype(bf), one_s.astype(bf)], axis=-1)
    in_maps = []
    for c in range(NCORES):
        sl = slice(c * NP, (c + 1) * NP)
        in_maps.append({
            "qt": np.ascontiguousarray(qt[sl]),
            "kt": np.ascontiguousarray(kt[sl]),
            "va": np.ascontiguousarray(va[sl]),
        })
    return in_maps


def _gather(results):
    outs = np.stack([results[c]["out"] for c in range(NCORES)])  # [8,NP,L,D]
    out = outs.reshape(B, H, L, D).transpose(0, 2, 1, 3)
    return np.ascontiguousarray(out)


def run_sharded(queries, keys, values, **kw):
    """Run on the 8 neuron cores; returns (full_output, BassKernelResults)."""
    nc = _build()
    in_maps = _prep_inputs(queries, keys, values)
    res = run_bass_kernel_spmd(nc, in_maps, list(range(NCORES)), **kw)
    return _gather(res.results), res


def kernel(queries, keys, values):
    out, _ = run_sharded(queries, keys, values)
    return out


# revision 39
# speedup vs baseline: 1.0160x; 1.0014x over previous
"""Trainium2 Bass kernel for DynamicSparseAttention.

Reference computation (per batch b, head h):
    scores  = Q @ K^T                      [L, S]
    dense   = softmax(scores, axis=-1)
    routing = dense ** 5
    combined = (routing + dense) * 0.5
    sparse  = combined / sum(combined, -1, keepdims=True)
    out     = sparse @ V                   [L, D]

Math: let p = exp(s - m1) with a per-row analytic upper bound m1 and
Z = sum_s p.  d = p/Z is the exact softmax, and with V' = [V | 1]:
    num = D5 @ V' + (P @ V')/Z   (rows 0..63 numerator, row 64 denominator)
    out = num[:, :D] / num[:, D]

Round A (per l-half): one fp16 score matmul stream (64 q/k channels plus a
ones*(-m1) shift channel, fp32 PSUM accumulate), exp on ACT -> p (bf16,
kept in SBUF), and B = P @ V' accumulated on the PE (ones column gives Z).

Round B builds p5 = d^5 per 128-row s-tile via two engine paths, balancing
the ACT and DVE engines (the two all-stock bottlenecks):
  - DVE path (N_DVE tiles):  d = p*zb (zb = broadcast bf16 1/Z), then
    d2 = d*d, d4 = d2*d2, p5 = d4*d  -- four 2x-mode tensor_tensor ops.
  - ACT path (rest): recompute scores with one extra channel
    (-1 ones row) * (lnZ row written on-device by an ACT Ln, fp16), then
    p5 = exp(5 * (s - m1 - lnZ)) in a single ACT pass.
Both produce the same d^5 scale, accumulated into A5 = P5 @ V'.

zb = broadcast bf16(1/Z) is exp(-lnZ) on ACT (a DVE reciprocal over
[1,1024] is an 8-pass iterative divide, ~7us) replicated across partitions
by a K=1 PE matmul against a ones row.

Epilogue: num = A5 + B*(1/Z) (two DVE tensor_tensor ops straight from
PSUM/SBUF), then per 128 rows: PE transpose, DVE reciprocal of the
denominator, ACT scaled copy, DMA out.

The (pair, l-half) units are software-pipelined depth 1, emitted as
  [B_act(i-1) | A(i) | Ln/zr/b_sb(i) | B_rest_mms(i-1) | zp(i) | B_epi(i-1)]
so each engine queue stays busy: the exp5s of unit i-1 sit ahead of unit
i's exps on ACT; the DVE chains of i-1 run during A(i)'s PE/ACT work and
are not queued behind mid(i)'s zb cast (which waits on late-A(i) PE work);
the zp replicate matmuls cover the PE's wait on the num add; and the
epilogue transposes allocate from the ps_acc ring (whose two slots are both
dead by then) instead of the score ring, which otherwise serializes the
next unit's score matmuls against this unit's epilogue.  Within A, PV-B for
s-tile st is emitted after scores+exp of st+1 so the PE never head-blocks
on an exp.  The last unit's round B is mostly-ACT (its DVE chains would
drain serially with no next unit to overlap).  PSUM: 2 score buffers
(4 banks) + 2 accumulators (4 banks) = 8 banks exactly; Bacc is freed
early via an ACT copy to SBUF (GPSIMD cannot read PSUM).

Numerics: rel err ~4.6e-3 on hardware (gate 2e-2).

Sharding: B*H = 32 (b,h) pairs, 4 per core across 8 cores, no cross-core
communication.  kernel() takes full inputs and returns the full output.
Q/K are pre-transposed on the host and DMA'd in [66, L] layout.
"""

import os
import sys
import numpy as np

for _p in ("/opt/trn_rl_repo",):
    if os.path.isdir(_p) and _p not in sys.path:
        sys.path.insert(0, _p)

from contextlib import ExitStack

import json as _json

import ml_dtypes

import concourse.bass as bass
import concourse.mybir as mybir
import concourse.tile as tile
import concourse.bass2jax as _bass2jax
import concourse.bass_utils as _bass_utils
from concourse.bass_utils import run_bass_kernel_spmd
from concourse.masks import make_identity

# ---------------------------------------------------------------------------
# Workaround: this container's walrus build rejects instructions carrying
# more than one sync wait ("Too many sync wait commands").  Tile's scheduler
# freely attaches 2-3 waits per instruction.  Rewrite the BIR JSON before
# compilation: excess waits are hoisted onto freshly inserted same-engine
# NoOp instructions placed immediately before the instruction, one wait
# each.  Semantics are unchanged (waits are conjunctive >= conditions and
# engine program order is preserved).
# ---------------------------------------------------------------------------

_MAX_WAITS = 1


def _split_waits_in_bir(bir_json: bytes) -> bytes:
    bir = _json.loads(bir_json)
    n_new = [0]

    def fix_block(bb):
        out = []
        for inst in bb["instructions"]:
            si = inst.get("sync_info") or {}
            waits = si.get("on_wait") or []
            if len(waits) > _MAX_WAITS:
                excess, keep = waits[:-_MAX_WAITS], waits[-_MAX_WAITS:]
                for w in excess:
                    n_new[0] += 1
                    out.append({
                        "debug": inst.get("debug", 0),
                        "engine": inst["engine"],
                        "ins": [],
                        "name": "I-wsplit-%d" % n_new[0],
                        "opcode": "NoOp",
                        "outs": [],
                        "sync_info": {"on_update": [], "on_wait": [w]},
                    })
                si["on_wait"] = keep
            out.append(inst)
        bb["instructions"] = out

    for fn in bir["functions"]:
        for bb in fn["blocks"]:
            fix_block(bb)
    return _json.dumps(bir).encode()


_orig_compile_bir_kernel = _bass_utils.compile_bir_kernel


def _patched_compile_bir_kernel(bir_json, tmpdir, neff_name="file.neff"):
    return _orig_compile_bir_kernel(
        _split_waits_in_bir(bir_json), tmpdir, neff_name=neff_name
    )


_bass_utils.compile_bir_kernel = _patched_compile_bir_kernel
_bass2jax.compile_bir_kernel = _patched_compile_bir_kernel

# (walrus's --enable-ldw-opt dedup is incompatible with the framework's
# explicit InstLdweights preamble on this build; leave it off.)

# ---------------------------------------------------------------------------

B, L, S, H, E, D = 2, 2048, 2048, 16, 64, 64
NCORES = 8
NP = (B * H) // NCORES  # pairs per core = 4
EC = E + 2   # channels: 64 fp16 q/k + ones*(-m1) + (-1)*lnZ
DV = D + 1   # v columns: 64 data + ones column (carries Z / denominator)
LHALF = 1024
NCH = 2      # 512-wide matmul chunks (PSUM bank limit)
NLH = L // LHALF
ST = S // 128
N_DVE = 10   # s-tiles per l-half whose p5 is computed on the DVE
FACTOR = 5.0

F32 = mybir.dt.float32
BF16 = mybir.dt.bfloat16
FP16 = mybir.dt.float16
EXP = mybir.ActivationFunctionType.Exp
LN = mybir.ActivationFunctionType.Ln
COPY = mybir.ActivationFunctionType.Copy
MULT = mybir.AluOpType.mult
ADD = mybir.AluOpType.add

M_COEF = float(np.sqrt(2.0 * np.log(S)))
M_MARGIN = 25.0


def _emit(ctx: ExitStack, tc: tile.TileContext, qtd, ktd, vad, outp):
    nc = tc.nc

    const = ctx.enter_context(tc.tile_pool(name="const", bufs=1))
    kq = ctx.enter_context(tc.tile_pool(name="kq", bufs=2))
    vpool = ctx.enter_context(tc.tile_pool(name="vp", bufs=2))
    ppool = ctx.enter_context(tc.tile_pool(name="pp", bufs=2))
    p5pool = ctx.enter_context(tc.tile_pool(name="p5", bufs=10))
    dpool = ctx.enter_context(tc.tile_pool(name="dp", bufs=3))
    zpool = ctx.enter_context(tc.tile_pool(name="zp", bufs=2))
    eppool = ctx.enter_context(tc.tile_pool(name="ep", bufs=3))
    opool = ctx.enter_context(tc.tile_pool(name="op", bufs=8))

    ps_sc = ctx.enter_context(tc.tile_pool(name="ps_sc", bufs=2, space="PSUM"))
    ps_acc = ctx.enter_context(tc.tile_pool(name="ps_acc", bufs=2, space="PSUM"))

    ident65 = const.tile([DV, DV], F32)
    make_identity(nc, ident65)
    identb65 = const.tile([DV, DV], BF16)
    make_identity(nc, identb65)
    ones_row = const.tile([1, 128], BF16)
    nc.vector.memset(ones_row, 1.0)

    # per-pair tiles, loaded once (ring 2 across pairs)
    loaded = {}

    def load_pair(bh):
        qtT = kq.tile([EC, L], FP16, tag="qt", name="qtT")
        nc.sync.dma_start(out=qtT, in_=qtd[bh])
        ktT = kq.tile([EC, S], FP16, tag="kt", name="ktT")
        nc.sync.dma_start(out=ktT, in_=ktd[bh])
        vts = []
        for t in range(ST):
            vt = vpool.tile([128, DV], BF16, tag=f"v{t}", name=f"vt{t}")
            nc.sync.dma_start(out=vt, in_=vad[bh, t * 128:(t + 1) * 128, :])
            vts.append(vt)
        loaded[bh] = (qtT, ktT, vts)

    def emit_A(bh, lh):
        qtT, ktT, vts = loaded[bh]
        l0 = lh * LHALF
        bacc = ps_acc.tile([DV, LHALF], F32, tag="acc", name="bacc")
        ps = []

        def emit_pvb(st):
            for c in range(NCH):
                cs = slice(c * 512, (c + 1) * 512)
                nc.tensor.matmul(bacc[:, cs], lhsT=vts[st], rhs=ps[st][:, cs],
                                 start=(st == 0), stop=(st == ST - 1))

        # PV-B for s-tile st is emitted after the scores+exp of st+1, so the
        # PE never head-blocks on the exp it just requested
        for st in range(ST):
            sb = slice(st * 128, (st + 1) * 128)
            sc = ps_sc.tile([128, LHALF], F32, tag="sc", name="scA")
            for c in range(NCH):
                cs = slice(c * 512, (c + 1) * 512)
                gs = slice(l0 + c * 512, l0 + (c + 1) * 512)
                nc.tensor.matmul(sc[:, cs], lhsT=ktT[0:E + 1, sb],
                                 rhs=qtT[0:E + 1, gs], start=True, stop=True)
            p = ppool.tile([128, LHALF], BF16, tag=f"p{st}", name="p")
            nc.scalar.activation(p, sc, EXP, bias=0.0, scale=1.0)
            ps.append(p)
            if st > 0:
                emit_pvb(st - 1)
        emit_pvb(ST - 1)

        # lnZ channel (fp16 row 65 of qt, matched with -1 ones row in kt).
        # Engine writes must start at a 32-aligned partition, so Ln lands in
        # an aligned scratch row and a tiny SBUF->SBUF DMA moves it into place.
        lz = zpool.tile([1, LHALF], FP16, tag="lz", name="lz")
        nc.scalar.activation(lz, bacc[D:DV, :], LN, bias=0.0, scale=1.0)
        nc.sync.dma_start(out=qtT[E + 1:E + 2, l0:l0 + LHALF], in_=lz)
        # zb = broadcast bf16(1/Z); 1/Z = exp(-lnZ) on ACT (a DVE reciprocal
        # over [1,1024] is an 8-pass iterative divide, ~7us -- far slower)
        zr = zpool.tile([1, LHALF], BF16, tag="zr", name="zr")
        nc.scalar.activation(zr, lz, EXP, bias=0.0, scale=-1.0)
        zp = ps_sc.tile([128, LHALF], F32, tag="sc", name="zp")
        for c in range(NCH):
            cs = slice(c * 512, (c + 1) * 512)
            nc.tensor.matmul(zp[:, cs], lhsT=ones_row, rhs=zr[:, cs],
                             start=True, stop=True)
        zb = zpool.tile([128, LHALF], BF16, tag="zb", name="zb")
        nc.vector.tensor_copy(zb, zp)
        # free bacc early (GPSIMD cannot read PSUM, so ACT does the copy)
        b_sb = eppool.tile([DV, LHALF], F32, tag="bsb", name="b_sb")
        nc.scalar.activation(b_sb, bacc, COPY, bias=0.0, scale=1.0)
        return {"ps": ps, "zb": zb, "b_sb": b_sb, "lh": lh, "bh": bh}

    N_ACT = ST - N_DVE

    def emit_B_act(stA):
        """Round-B ACT-path tiles (sts 0..n_act-1): score recompute with the
        lnZ channel + exp(5*) on ACT.  Emitted BEFORE the next unit's A-phase
        so the exp5s sit early in the ACT queue."""
        bh, lh = stA["bh"], stA["lh"]
        qtT, ktT, vts = loaded[bh]
        l0 = lh * LHALF
        a5 = ps_acc.tile([DV, LHALF], F32, tag="acc", name="a5")
        stA["a5"] = a5
        # t1 = B/Z only needs b_sb and zb (ready since this unit's mid);
        # computing it here keeps the epilogue's num = t1 + A5 one DVE op
        # after the last chain instead of two
        t1 = eppool.tile([DV, LHALF], F32, tag="t1", name="t1")
        nc.vector.tensor_tensor(out=t1, in0=stA["b_sb"], in1=stA["zb"][0:DV, :],
                                op=MULT)
        stA["t1"] = t1
        p5s = {}

        def emit_a5(st):
            for c in range(NCH):
                cs = slice(c * 512, (c + 1) * 512)
                nc.tensor.matmul(a5[:, cs], lhsT=vts[st], rhs=p5s[st][:, cs],
                                 start=(st == 0), stop=(st == ST - 1))

        for st in range(stA["n_act"]):
            sb = slice(st * 128, (st + 1) * 128)
            sc = ps_sc.tile([128, LHALF], F32, tag="sc", name="scB")
            for c in range(NCH):
                cs = slice(c * 512, (c + 1) * 512)
                gs = slice(l0 + c * 512, l0 + (c + 1) * 512)
                nc.tensor.matmul(sc[:, cs], lhsT=ktT[:, sb],
                                 rhs=qtT[:, gs], start=True, stop=True)
            p5 = p5pool.tile([128, LHALF], BF16, tag="p5a", name="p5")
            nc.scalar.activation(p5, sc, EXP, bias=0.0, scale=FACTOR)
            p5s[st] = p5
            if st > 0:
                emit_a5(st - 1)
        emit_a5(stA["n_act"] - 1)

    def emit_B_rest(stA):
        """Round-B DVE-path tiles (sts N_ACT..15) + combine + epilogue."""
        bh, lh = stA["bh"], stA["lh"]
        qtT, ktT, vts = loaded[bh]
        ps, zb, b_sb, a5 = stA["ps"], stA["zb"], stA["b_sb"], stA["a5"]
        l0 = lh * LHALF
        for st in range(stA["n_act"], ST):
            d = dpool.tile([128, LHALF], BF16, tag="d", name="d")
            nc.vector.tensor_tensor(out=d, in0=ps[st], in1=zb, op=MULT)
            d2 = dpool.tile([128, LHALF], BF16, tag="d2", name="d2")
            nc.vector.tensor_tensor(out=d2, in0=d, in1=d, op=MULT)
            d4 = dpool.tile([128, LHALF], BF16, tag="d4", name="d4")
            nc.vector.tensor_tensor(out=d4, in0=d2, in1=d2, op=MULT)
            p5 = p5pool.tile([128, LHALF], BF16, tag="p5", name="p5")
            nc.vector.tensor_tensor(out=p5, in0=d4, in1=d, op=MULT)
            for c in range(NCH):
                cs = slice(c * 512, (c + 1) * 512)
                nc.tensor.matmul(a5[:, cs], lhsT=vts[st], rhs=p5[:, cs],
                                 start=False, stop=(st == ST - 1))

        # combine: num = A5 + B/Z (one DVE op; t1 was computed in B_act)
        t1 = stA["t1"]
        num = eppool.tile([DV, LHALF], F32, tag="num", name="num")
        nc.vector.tensor_tensor(out=num, in0=t1, in1=a5, op=ADD)
        stA["num"] = num

    def emit_B_epi(stA):
        """out = num[:, :D] / num[:, D].  The nt transposes allocate from the
        ps_acc ring (both of its slots are dead here: bacc freed by the early
        b_sb copy, a5 by the num add) so the sc ring is left free for the
        next unit's score tiles."""
        bh, lh, num = stA["bh"], stA["lh"], stA["num"]
        l0 = lh * LHALF
        for ch in range(LHALF // 128):
            nt = ps_acc.tile([128, DV], F32, tag="acc", name="nt")
            nc.tensor.transpose(nt, num[:, ch * 128:(ch + 1) * 128], ident65)
            rz = opool.tile([128, 1], F32, tag="rz", name="rz")
            nc.vector.reciprocal(rz, nt[:, D:DV])
            ot = opool.tile([128, D], F32, tag="ot", name="ot")
            nc.scalar.activation(ot, nt[:, 0:D], COPY, bias=0.0,
                                 scale=rz[:, 0:1])
            lrow = l0 + ch * 128
            nc.gpsimd.dma_start(out=outp[bh, lrow:lrow + 128, :], in_=ot)

    units = [(bh, lh) for bh in range(NP) for lh in range(NLH)]
    pending = None
    for i, (bh, lh) in enumerate(units):
        if lh == 0:
            load_pair(bh)
        if pending is not None:
            emit_B_act(pending)
        qtT, ktT, vts = loaded[bh]
        l0 = lh * LHALF
        bacc = ps_acc.tile([DV, LHALF], F32, tag="acc", name="bacc")
        ps = []

        def emit_pvb(st):
            for c in range(NCH):
                cs = slice(c * 512, (c + 1) * 512)
                nc.tensor.matmul(bacc[:, cs], lhsT=vts[st], rhs=ps[st][:, cs],
                                 start=(st == 0), stop=(st == ST - 1))

        # PV-B for s-tile st is emitted after the scores+exp of st+1, so the
        # PE never head-blocks on the exp it just requested
        for st in range(ST):
            sb = slice(st * 128, (st + 1) * 128)
            sc = ps_sc.tile([128, LHALF], F32, tag="sc", name="scA")
            for c in range(NCH):
                cs = slice(c * 512, (c + 1) * 512)
                gs = slice(l0 + c * 512, l0 + (c + 1) * 512)
                nc.tensor.matmul(sc[:, cs], lhsT=ktT[0:E + 1, sb],
                                 rhs=qtT[0:E + 1, gs], start=True, stop=True)
            p = ppool.tile([128, LHALF], BF16, tag=f"p{st}", name="p")
            nc.scalar.activation(p, sc, EXP, bias=0.0, scale=1.0)
            ps.append(p)
            if st > 0:
                emit_pvb(st - 1)
        emit_pvb(ST - 1)

        # lnZ chain early: Ln gates the lnZ DMA and thus every B_act(i) PE
        # matmul; emitting it here puts it ahead of the previous unit's
        # epilogue copies in the ACT queue.
        lz = zpool.tile([1, LHALF], FP16, tag="lz", name="lz")
        nc.scalar.activation(lz, bacc[D:DV, :], LN, bias=0.0, scale=1.0)
        nc.sync.dma_start(out=qtT[E + 1:E + 2, l0:l0 + LHALF], in_=lz)
        zr = zpool.tile([1, LHALF], BF16, tag="zr", name="zr")
        nc.scalar.activation(zr, lz, EXP, bias=0.0, scale=-1.0)
        # free bacc now: as the LAST ACT op of a cycle this stalled both the
        # next a5 ring allocation (PE) and the hoisted t1 (DVE head-of-line)
        b_sb = eppool.tile([DV, LHALF], F32, tag="bsb", name="b_sb")
        nc.scalar.activation(b_sb, bacc, COPY, bias=0.0, scale=1.0)

        # B_rest(i-1): its DVE chains are ready (zb of i-1 exists) and must
        # not queue behind mid(i)'s zb cast, which waits on late PE work.
        if pending is not None:
            emit_B_rest(pending)

        # ---- mid: zb broadcast.  The zp matmuls sit between B_rest(i-1)'s
        # A5 matmuls and its epilogue transposes in the PE queue, covering
        # the PE's wait on the num add (DVE).
        zp = ps_sc.tile([128, LHALF], F32, tag="sc", name="zp")
        for c in range(NCH):
            cs = slice(c * 512, (c + 1) * 512)
            nc.tensor.matmul(zp[:, cs], lhsT=ones_row, rhs=zr[:, cs],
                             start=True, stop=True)
        zb = zpool.tile([128, LHALF], BF16, tag="zb", name="zb")
        nc.vector.tensor_copy(zb, zp)

        if pending is not None:
            emit_B_epi(pending)

        # the last unit's round B drains with no A-phase to overlap; give it
        # all-ACT tiles so its p5 work overlaps its own PE/ACT instead of
        # serializing on the DVE chain
        # last unit: no next unit overlaps its round B, so balance its p5
        # work across ACT (exp5) and DVE (chains) instead of one engine
        n_act = N_ACT if i < len(units) - 1 else 10
        pending = {"ps": ps, "zb": zb, "b_sb": b_sb, "lh": lh, "bh": bh,
                   "n_act": n_act}
    emit_B_act(pending)
    emit_B_rest(pending)
    emit_B_epi(pending)


_CACHE = {}


def _build():
    if "nc" in _CACHE:
        return _CACHE["nc"]
    nc = bass.Bass()
    qtd = nc.declare_dram_parameter("qt", [NP, EC, L], FP16, isOutput=False)
    ktd = nc.declare_dram_parameter("kt", [NP, EC, S], FP16, isOutput=False)
    vad = nc.declare_dram_parameter("va", [NP, S, DV], BF16, isOutput=False)
    outp = nc.declare_dram_parameter("out", [NP, L, D], F32, isOutput=True)
    with tile.TileContext(nc) as tc:
        with ExitStack() as ctx:
            _emit(ctx, tc, qtd[:], ktd[:], vad[:], outp[:])
    _CACHE["nc"] = nc
    return nc


def _prep_inputs(queries, keys, values):
    bf = ml_dtypes.bfloat16
    q = np.ascontiguousarray(np.asarray(queries, np.float32)
                             .transpose(0, 2, 1, 3)).reshape(B * H, L, E)
    k = np.ascontiguousarray(np.asarray(keys, np.float32)
                             .transpose(0, 2, 1, 3)).reshape(B * H, S, E)
    v = np.ascontiguousarray(np.asarray(values, np.float32)
                             .transpose(0, 2, 1, 3)).reshape(B * H, S, D)
    m1 = (M_COEF * np.sqrt((q.astype(np.float64) ** 2).sum(-1)) + M_MARGIN
          ).astype(np.float32)  # [BH, L]
    one_s = np.ones((B * H, S, 1), np.float32)
    zero_l = np.zeros((B * H, L, 1), np.float32)
    # qt channels: q | -m1 | lnZ placeholder (written on device per l-half)
    qt = np.concatenate([q, -m1[..., None], zero_l], axis=-1)   # [., L, 66]
    # kt channels: k | +1 (pairs with -m1) | -1 (pairs with lnZ)
    kt = np.concatenate([k, one_s, -one_s], axis=-1)            # [., S, 66]
    qt = np.ascontiguousarray(qt.transpose(0, 2, 1)).astype(np.float16)
    kt = np.ascontiguousarray(kt.transpose(0, 2, 1)).astype(np.float16)
    va = np.concatenate([v.astype(bf), one_s.astype(bf)], axis=-1)
    in_maps = []
    for c in range(NCORES):
        sl = slice(c * NP, (c + 1) * NP)
        in_maps.append({
            "qt": np.ascontiguousarray(qt[sl]),
            "kt": np.ascontiguousarray(kt[sl]),
            "va": np.ascontiguousarray(va[sl]),
        })
    return in_maps


def _gather(results):
    outs = np.stack([results[c]["out"] for c in range(NCORES)])  # [8,NP,L,D]
    out = outs.reshape(B, H, L, D).transpose(0, 2, 1, 3)
    return np.ascontiguousarray(out)


def run_sharded(queries, keys, values, **kw):
    """Run on the 8 neuron cores; returns (full_output, BassKernelResults)."""
    nc = _build()
    in_maps = _prep_inputs(queries, keys, values)
    res = run_bass_kernel_spmd(nc, in_maps, list(range(NCORES)), **kw)
    return _gather(res.results), res


def kernel(queries, keys, values):
    out, _ = run_sharded(queries, keys, values)
    return out
